# revision 12
# baseline (speedup 1.0000x reference)
"""ColorizationNet Trainium2 kernel (8 NeuronCores, SPMD, two phases).

Structure exploited: rows of the big FC input [4096, 32786] share an identical
x_conv prefix (32768 cols), so

    fc_in @ w1.T = x_conv @ w1[:, :32768].T  (one shared matvec, [304])
                 + [pos|chunks] @ w1[:, 32768:].T  ([4096,18] GEMM)

Sharding (core r of 8):
  - conv backbone row-sharded: core r produces the x_conv slice for pooled
    rows [4r, 4r+4) of every channel (halos via zero-padded input windows,
    out-of-image "phantom" rows masked to zero via activation scale).
  - shared matvec K-sharded to match (each core streams 1/8 of w1's big
    part as fp16, 2.5 MB).  Phase A outputs the 8 partials [304]; the host
    sums them (collectives work under the axon PJRT path but cost ~70 us,
    far more than the second NEFF launch they would save).
  - phase B: patch FC sharded by patch row, core r handles patches
    [512r, 512(r+1)).

Perf structure (vs the fp32 9-matmul-per-layer baseline, 82.8 us):
  - all conv/matvec/FC matmul operands fp16: moving operand streams at
    1 col/cycle at 2.4 GHz (fp32 needs 2 half-speed passes = 4x), and the
    w1 stream (phase A's DMA floor) halves to ~2.5 MB.
  - conv1's three 3x3 column taps are folded into the contraction dim
    (k=3*18=54) so each output block is ONE matmul; conv1/2/3 blocks are
    fused along the moving free dim -> 8 conv matmuls total instead of 27
    (per-matmul overhead ~250-400 ns dominates these small convs).
  - matvec k-chunks are ordered so passes 0-15 only need conv3's first
    output block (overlap with block 2), fp16 stream = 127 ns/pass.
  - DMA issue (~0.7 us of sequencer time per dma_start) spread across the
    sync/scalar/vector/gpsimd queues so descriptor generation overlaps.
"""

import sys

for _p in ("/opt/trn_rl_repo",):
    if _p not in sys.path:
        sys.path.insert(0, _p)

import numpy as np
from contextlib import ExitStack

IMG = 256
CS = 4
G = 64
H1 = 304
H2 = 176
OUT = 48
NCORES = 8

# phase-A fp16 packed stationaries: [96 partitions, 896]
#   s1f [54, 128] at cols [0, 128)   (dx-folded conv1)
#   s2  [80, 384] at cols [128, 512)
#   s3  [96, 384] at cols [512, 896)
CW16 = 896
# phase-A fp32 masks/biases: [64, 13]
#   mk1 b=0..2 @0..3, bm1 @3..6, mk2 @6..9, bm2 @9..12, bc3 @12
CW32 = 13

# phase-B fp16 packed consts: [128 partitions, 1440]
#   extrasT [18, 512] @0, w1eT [18, 304] @512, w2a/b [128, 176] @816/@992,
#   w2c [48, 176] @1168, w3a [128, 48] @1344, w3b [48, 48] @1392
CBW16 = 1440
# phase-B fp32 biases: [128, 3]: b2a [128] @0, b2b [48] @1, b3 [48] @2
CBW32 = 3


def _build_s1(c1_w):
    # [18, 3, 128]: rows i = in-row in window; cols m = s*64 + jp*8 + c
    s1 = np.zeros((18, 3, 128), np.float32)
    for dx in range(3):
        for s in range(2):
            for jp in range(8):
                j = 2 * jp + s
                for c in range(8):
                    m = s * 64 + jp * 8 + c
                    for dy in range(3):
                        s1[j + dy, dx, m] = c1_w[c, 0, dy, dx]
    return s1


def _build_s2(c2_w):
    # [80, 3, 128]: rows k = delta*8 + ci (ci in 0..8); cols m = s*64+jp*16+co
    s2 = np.zeros((80, 3, 128), np.float32)
    for dx in range(3):
        for s in range(2):
            for jp in range(4):
                j2 = 2 * jp + s
                for co in range(16):
                    m = s * 64 + jp * 16 + co
                    for ci in range(8):
                        for dy in range(3):
                            s2[(j2 + dy) * 8 + ci, dx, m] = c2_w[co, ci, dy, dx]
    return np.ascontiguousarray(s2.reshape(80, 3 * 128))


def _build_s3(c3_w):
    # [96, 3, 128]: rows k = delta*16 + ci (ci in 0..16); cols m = s*64+jpp*32+co
    s3 = np.zeros((96, 3, 128), np.float32)
    for dx in range(3):
        for s in range(2):
            for jpp in range(2):
                j3 = 2 * jpp + s
                for co in range(32):
                    m = s * 64 + jpp * 32 + co
                    for ci in range(16):
                        for dy in range(3):
                            s3[(j3 + dy) * 16 + ci, dx, m] = c3_w[co, ci, dy, dx]
    return np.ascontiguousarray(s3.reshape(96, 3 * 128))


def _host_inputs(x, c1_w, c1_b, c2_w, c2_b, c3_w, c3_b, w1, b1, w2, b2, w3, b3):
    """Returns (in_maps_a, in_maps_b_partial, b1). Each phase-A map has
    'c1mv' [54,768] f16, 'cw16' [96,896] f16, 'cw32' [64,13] f32,
    'w1ps' [128, 32*304] f16. Each phase-B map has 'cb16' [128,1440] f16 and
    'cb32' [128,3] f32; 'shc' [128,3] f32 is added after phase A."""
    f16 = np.float16
    x = np.asarray(x, np.float32).reshape(IMG, IMG)
    s1 = _build_s1(np.asarray(c1_w, np.float32))  # [18, 3, 128]
    s1f = np.ascontiguousarray(s1.transpose(1, 0, 2).reshape(54, 128))
    s2 = _build_s2(np.asarray(c2_w, np.float32))
    s3 = _build_s3(np.asarray(c3_w, np.float32))

    cw16 = np.zeros((96, CW16), f16)
    cw16[0:54, 0:128] = s1f
    cw16[0:80, 128:512] = s2
    cw16[0:96, 512:896] = s3

    # phase-B packed consts (same for every core except extrasT)
    cb16_0 = np.zeros((128, CBW16), f16)
    w1eT = np.asarray(w1, np.float32)[:, 32768:].T  # [18, 304]
    w2T = np.asarray(w2, np.float32).T  # [304, 176]
    w3T = np.asarray(w3, np.float32).T  # [176, 48]
    cb16_0[0:18, 512:816] = w1eT
    cb16_0[0:128, 816:992] = w2T[0:128]
    cb16_0[0:128, 992:1168] = w2T[128:256]
    cb16_0[0:48, 1168:1344] = w2T[256:304]
    cb16_0[0:128, 1344:1392] = w3T[0:128]
    cb16_0[0:48, 1392:1440] = w3T[128:176]
    cb32 = np.zeros((128, CBW32), np.float32)
    cb32[0:128, 0] = np.asarray(b2, np.float32)[0:128]
    cb32[0:48, 1] = np.asarray(b2, np.float32)[128:176]
    cb32[0:48, 2] = np.asarray(b3, np.float32)

    w1bigT = np.ascontiguousarray(np.asarray(w1, np.float32)[:, :32768].T)  # [32768, 304]
    chunks = x.reshape(G, CS, G, CS).transpose(0, 2, 1, 3).reshape(G * G, CS * CS)
    pi = (np.arange(G * G) // G).astype(np.float32) * CS
    pj = (np.arange(G * G) % G).astype(np.float32) * CS

    c1b = np.asarray(c1_b, np.float32)
    c2b = np.asarray(c2_b, np.float32)
    bc3v = np.tile(np.asarray(c3_b, np.float32), 2)  # [64]

    # matvec k-chunk layout: pass b = 16*g + b' uses conv3 output block g;
    # partition p = 64*hi + jpp*32 + co -> x_conv (co, 4r + 2g + jpp, 16*hi + b')
    P = np.arange(128)[:, None]
    B = np.arange(32)[None, :]
    co = P % 32
    jpp = (P % 64) // 32
    hi = P // 64
    g = B // 16
    bp = B % 16

    maps_a, maps_b = [], []
    for r in range(NCORES):
        # padded input strip: x rows [32r-7, 32r+43), cols padded by 1
        xpad = np.zeros((50, 258), np.float32)
        lo = 32 * r - 7
        hi_row = 32 * r + 43
        slo, shi = max(lo, 0), min(hi_row, IMG)
        xpad[slo - lo : shi - lo, 1:257] = x[slo:shi, :]

        # conv1 moving, dx folded into partitions, 3 row-blocks along free
        c1mv = np.zeros((54, 768), f16)
        for dx in range(3):
            for b in range(3):
                c1mv[dx * 18 : dx * 18 + 18, 256 * b : 256 * b + 256] = xpad[
                    16 * b : 16 * b + 18, dx : dx + 256
                ]

        cw32 = np.zeros((64, CW32), np.float32)
        for b in range(3):
            for jp in range(8):
                valid = 0 <= (16 * r - 3 + 8 * b + jp) < 128
                cw32[jp * 8 : jp * 8 + 8, 0 + b] = 1.0 if valid else 0.0
                cw32[jp * 8 : jp * 8 + 8, 3 + b] = c1b if valid else 0.0
            for jp in range(4):
                valid = 0 <= (8 * r - 1 + 4 * b + jp) < 64
                cw32[jp * 16 : jp * 16 + 16, 6 + b] = 1.0 if valid else 0.0
                cw32[jp * 16 : jp * 16 + 16, 9 + b] = c2b if valid else 0.0
        cw32[0:64, 12] = bc3v

        row = 4 * r + 2 * g + jpp
        kg = co * 1024 + row * 32 + 16 * hi + bp  # [128, 32]
        w1ps = w1bigT[kg.ravel()].reshape(128, 32 * 304).astype(f16)
        maps_a.append({"c1mv": c1mv, "cw16": cw16, "cw32": cw32, "w1ps": w1ps})

        cb16 = cb16_0.copy()
        sl = slice(512 * r, 512 * (r + 1))
        cb16[0, 0:512] = pi[sl]
        cb16[1, 0:512] = pj[sl]
        cb16[2:18, 0:512] = chunks[sl].T
        maps_b.append({"cb16": cb16, "cb32": cb32})
    return maps_a, maps_b, np.asarray(b1, np.float32)


def _mk_nc():
    import concourse.bacc as bacc

    # Bacc (not raw Bass): its compile() runs move_matmul_waits_to_ldweights /
    # generate_event_semaphores, required for the 1-wait-per-instruction
    # hardware constraint.
    return bacc.Bacc("TRN2", target_bir_lowering=False, debug=False, num_devices=NCORES)


def _build_phase_a():
    """Convs + sharded shared-matvec partial. Output: part [1, 304]."""
    import concourse.tile as tile
    from concourse import mybir

    f32 = mybir.dt.float32
    f16 = mybir.dt.float16
    AF = mybir.ActivationFunctionType
    nc = _mk_nc()

    c1mv_d = nc.dram_tensor("c1mv", [54, 768], f16, kind="ExternalInput").ap()
    cw16_d = nc.dram_tensor("cw16", [96, CW16], f16, kind="ExternalInput").ap()
    cw32_d = nc.dram_tensor("cw32", [64, CW32], f32, kind="ExternalInput").ap()
    w1ps_d = nc.dram_tensor("w1ps", [128, 32 * 304], f16, kind="ExternalInput").ap()
    part_d = nc.dram_tensor("part", [1, 304], f32, kind="ExternalOutput").ap()

    with tile.TileContext(nc) as tc, ExitStack() as ctx:
        cpool = ctx.enter_context(tc.tile_pool(name="consts", bufs=1))
        spool = ctx.enter_context(tc.tile_pool(name="work", bufs=1))
        pconv = ctx.enter_context(tc.tile_pool(name="pconv", bufs=1, space="PSUM"))
        pmv = ctx.enter_context(tc.tile_pool(name="pmv", bufs=1, space="PSUM"))

        # warm the ScalarE activation-function table early (overlaps DMAs)
        scr = cpool.tile([1, 1], f32, tag="scr")
        nc.vector.memset(scr[:], 0.0)
        scr2 = cpool.tile([1, 1], f32, tag="scr2")
        nc.scalar.activation(scr2[:], scr[:], AF.Relu)

        # parallel-issue the input DMAs; keep the ScalarE queue free of DMA
        # descriptor generation (it stalls the conv RELU writes otherwise)
        cw16_t = cpool.tile([96, CW16], f16, tag="cw16")
        nc.sync.dma_start(cw16_t[:], cw16_d)
        c1mv_t = cpool.tile([54, 768], f16, tag="c1mv")
        nc.sync.dma_start(c1mv_t[:], c1mv_d)
        cw32_t = cpool.tile([64, CW32], f32, tag="cw32")
        nc.scalar.dma_start(cw32_t[:], cw32_d)
        wst = cpool.tile([128, 32 * 304], f16, tag="w1s")
        half = 16 * 304
        nc.gpsimd.dma_start(wst[:, 0:half], w1ps_d[:, 0:half])
        nc.gpsimd.dma_start(wst[:, half : 2 * half], w1ps_d[:, half : 2 * half])

        # PE p-state warm-up: the PE only reaches 2.4 GHz after ~3 us of
        # continuous execution and drops back when idle, so dependency-free
        # dummy matmuls bridge the DMA wait and the pool/activation gaps.
        dum = cpool.tile([128, 256], f16, tag="dum")
        nc.gpsimd.memset(dum[:], 0.0)
        dps = pmv.tile([1, 256], f32, tag="dps")

        def warm(n):
            for _ in range(n):
                nc.tensor.matmul(
                    dps[:], lhsT=dum[:, 0:1], rhs=dum[:], start=True, stop=True
                )

        warm(14)

        s1f = cw16_t[0:54, 0:128]
        s2ap = lambda dx: cw16_t[0:80, 128 + 128 * dx : 256 + 128 * dx]
        s3ap = lambda dx: cw16_t[0:96, 512 + 128 * dx : 640 + 128 * dx]
        mk1 = lambda b, n=64: cw32_t[0:n, 0 + b : 1 + b]
        bm1 = lambda b, n=64: cw32_t[0:n, 3 + b : 4 + b]
        mk2 = lambda b, n=64: cw32_t[0:n, 6 + b : 7 + b]
        bm2 = lambda b, n=64: cw32_t[0:n, 9 + b : 10 + b]
        bc3 = cw32_t[0:64, 12:13]

        # next-layer moving-window tiles (built in place by ScalarE writes)
        m2all = cpool.tile([80, 390], f16, tag="m2all")  # 3 x (1+128+1)
        m3all = cpool.tile([96, 132], f16, tag="m3all")  # 2 x (1+64+1)
        xc_t = cpool.tile([128, 32], f16, tag="xc")
        nc.vector.memset(m2all[:], 0.0)
        nc.vector.memset(m3all[:], 0.0)

        def vpool(ps, width, tag):
            """psum [128, width] (partition = (s, *)) -> [64, width] s-max."""
            vtop = spool.tile([64, width], f32, tag=f"vt{tag}")
            nc.scalar.copy(vtop[:], ps[0:64, :])
            v = spool.tile([64, width], f32, tag=f"v{tag}")
            nc.vector.tensor_max(v[:], ps[64:128, :], vtop[:])
            return v

        def hpool(v, width, tag):
            """[64, width] -> [64, width//2] adjacent-pair max."""
            vv = v[:].rearrange("p (x t) -> p x t", t=2)
            ph = spool.tile([64, width // 2], f32, tag=f"ph{tag}")
            nc.vector.tensor_max(ph[:], vv[:, :, 0], vv[:, :, 1])
            return ph

        # ---- conv1: dx folded in k; blocks 0,1 fused (n=512) + block 2
        ps1a = pconv.tile([128, 512], f32, tag="ps1a")
        nc.tensor.matmul(ps1a[:], lhsT=s1f, rhs=c1mv_t[:, 0:512], start=True, stop=True)
        ps1b = pconv.tile([128, 256], f32, tag="ps1b")
        nc.tensor.matmul(ps1b[:], lhsT=s1f, rhs=c1mv_t[:, 512:768], start=True, stop=True)
        ph1a = hpool(vpool(ps1a, 512, "1a"), 512, "1a")  # [64, 256] = (b 2, x 128)
        ph1b = hpool(vpool(ps1b, 256, "1b"), 256, "1b")  # [64, 128]
        A = nc.scalar.activation
        A(m2all[0:64, 1:129], ph1a[:, 0:128], AF.Relu, bias=bm1(0), scale=mk1(0))
        A(m2all[0:64, 131:259], ph1a[:, 128:256], AF.Relu, bias=bm1(1), scale=mk1(1))
        A(m2all[0:64, 261:389], ph1b[:], AF.Relu, bias=bm1(2), scale=mk1(2))
        A(m2all[64:80, 1:129], ph1a[0:16, 128:256], AF.Relu, bias=bm1(1, 16), scale=mk1(1, 16))
        A(m2all[64:80, 131:259], ph1b[0:16, :], AF.Relu, bias=bm1(2, 16), scale=mk1(2, 16))

        warm(18)  # bridge pool1 + m2all activation writes

        # ---- conv2: 3 dx-matmuls, blocks fused (n=384)
        ps2 = pconv.tile([128, 384], f32, tag="ps2")
        m2v = m2all[:].rearrange("p (b c) -> p b c", b=3)
        for dx in range(3):
            nc.tensor.matmul(
                ps2[:], lhsT=s2ap(dx), rhs=m2v[:, :, dx : dx + 128],
                start=(dx == 0), stop=(dx == 2),
            )
        ph2 = hpool(vpool(ps2, 384, "2"), 384, "2")  # [64, 192] = (b 3, x 64)
        A(m3all[0:64, 1:65], ph2[:, 0:64], AF.Relu, bias=bm2(0), scale=mk2(0))
        A(m3all[0:64, 67:131], ph2[:, 64:128], AF.Relu, bias=bm2(1), scale=mk2(1))
        A(m3all[64:96, 1:65], ph2[0:32, 64:128], AF.Relu, bias=bm2(1, 32), scale=mk2(1, 32))
        A(m3all[64:96, 67:131], ph2[0:32, 128:192], AF.Relu, bias=bm2(2, 32), scale=mk2(2, 32))

        warm(12)  # bridge pool2 + m3all activation writes

        # ---- conv3: 3 dx-matmuls, g-blocks fused (n=128)
        ps3 = pconv.tile([128, 128], f32, tag="ps3")
        m3v = m3all[:].rearrange("p (g c) -> p g c", g=2)
        for dx in range(3):
            nc.tensor.matmul(
                ps3[:], lhsT=s3ap(dx), rhs=m3v[:, :, dx : dx + 64],
                start=(dx == 0), stop=(dx == 2),
            )
        ph3 = hpool(vpool(ps3, 128, "3"), 128, "3")  # [64, 64] = (g 2, x 32)
        # xc[64*hi:, 16g:16g+16] = relu(ph3[:, 32g+16hi : +16] + bc3)
        A(xc_t[0:64, 0:16], ph3[:, 0:16], AF.Relu, bias=bc3)
        A(xc_t[64:128, 0:16], ph3[:, 16:32], AF.Relu, bias=bc3)
        A(xc_t[0:64, 16:32], ph3[:, 32:48], AF.Relu, bias=bc3)
        A(xc_t[64:128, 16:32], ph3[:, 48:64], AF.Relu, bias=bc3)

        warm(7)  # bridge pool3 + xc activation writes

        # ---- shared matvec partial [1, 304]; passes 0-15 need only g=0
        ps_mv = pmv.tile([1, 304], f32, tag="mv")
        for b in range(32):
            nc.tensor.matmul(
                ps_mv[:],
                lhsT=xc_t[:, b : b + 1],
                rhs=wst[:, 304 * b : 304 * (b + 1)],
                start=(b == 0),
                stop=(b == 31),
            )
        part_s = spool.tile([1, 304], f32, tag="part")
        nc.scalar.copy(part_s[:], ps_mv[:])
        nc.sync.dma_start(part_d, part_s[:])

    nc.compile()
    return nc


def _build_phase_b():
    """Patch FC for this core's 512 patches, given summed shared vector."""
    import concourse.tile as tile
    from concourse import mybir

    f32 = mybir.dt.float32
    f16 = mybir.dt.float16
    AF = mybir.ActivationFunctionType
    ALU = mybir.AluOpType
    nc = _mk_nc()

    cb16_d = nc.dram_tensor("cb16", [128, CBW16], f16, kind="ExternalInput").ap()
    cb32_d = nc.dram_tensor("cb32", [128, CBW32], f32, kind="ExternalInput").ap()
    shc_d = nc.dram_tensor("shc", [128, 3], f32, kind="ExternalInput").ap()
    yout_d = nc.dram_tensor("yout", [48, 512], f32, kind="ExternalOutput").ap()

    mblk = [(0, 128), (128, 128), (256, 48)]
    qblk = [(0, 128), (128, 48)]

    with tile.TileContext(nc) as tc, ExitStack() as ctx:
        cpool = ctx.enter_context(tc.tile_pool(name="consts", bufs=1))
        fpool = ctx.enter_context(tc.tile_pool(name="fc", bufs=1))
        pfc = ctx.enter_context(tc.tile_pool(name="pfc", bufs=1, space="PSUM"))
        phh = ctx.enter_context(tc.tile_pool(name="phh", bufs=3, space="PSUM"))

        # warm the ScalarE Sigmoid table early (overlaps DMAs); ReLUs are on DVE
        scr = cpool.tile([1, 1], f32, tag="scr")
        nc.vector.memset(scr[:], 0.0)
        scr2 = cpool.tile([1, 1], f32, tag="scr2")
        nc.scalar.activation(scr2[:], scr[:], AF.Sigmoid)

        cb = cpool.tile([128, CBW16], f16, tag="cb16")
        nc.sync.dma_start(cb[:, 0:816], cb16_d[:, 0:816])
        shc = cpool.tile([128, 3], f32, tag="shc")
        nc.scalar.dma_start(shc[:], shc_d)
        cbb = cpool.tile([128, CBW32], f32, tag="cb32")
        nc.scalar.dma_start(cbb[:], cb32_d)
        nc.gpsimd.dma_start(cb[:, 816:CBW16], cb16_d[:, 816:CBW16])

        # PE p-state warm-up while the const DMA lands (see phase A)
        dum = cpool.tile([128, 256], f16, tag="dum")
        nc.gpsimd.memset(dum[:], 0.0)
        dps = pfc.tile([1, 256], f32, tag="dps")
        for _ in range(16):
            nc.tensor.matmul(dps[:], lhsT=dum[:, 0:1], rhs=dum[:], start=True, stop=True)

        extrasT = cb[0:18, 0:512]
        w1eT = cb[0:18, 512:816]
        w2T_t = [cb[0:128, 816:992], cb[0:128, 992:1168], cb[0:48, 1168:1344]]
        w3T_t = [cb[0:128, 1344:1392], cb[0:48, 1392:1440]]
        b2c_t = [cbb[0:128, 0:1], cbb[0:48, 1:2]]
        b3c_t = cbb[0:48, 2:3]
        sh_t = [shc[0:128, 0:1], shc[0:128, 1:2], shc[0:48, 2:3]]

        h1_t = []
        for i, (off, mb) in enumerate(mblk):
            ps_e = pfc.tile([mb, 512], f32, tag=f"pse{i}")
            nc.tensor.matmul(
                ps_e[:], lhsT=w1eT[:, off : off + mb], rhs=extrasT, start=True, stop=True
            )
            h1 = fpool.tile([mb, 512], f16, tag=f"h1{i}")
            nc.vector.tensor_scalar(h1[:], ps_e[:], sh_t[i], 0.0, ALU.add, ALU.max)
            h1_t.append(h1)

        h2_t = []
        for q, (qoff, mq) in enumerate(qblk):
            ps_h = phh.tile([mq, 512], f32, tag="psh")
            for i, (off, mb) in enumerate(mblk):
                nc.tensor.matmul(
                    ps_h[:],
                    lhsT=w2T_t[i][:, qoff : qoff + mq],
                    rhs=h1_t[i][:],
                    start=(i == 0),
                    stop=(i == 2),
                )
            h2 = fpool.tile([mq, 512], f16, tag=f"h2{q}")
            nc.vector.tensor_scalar(h2[:], ps_h[:], b2c_t[q], 0.0, ALU.add, ALU.max)
            h2_t.append(h2)

        ps_o = phh.tile([48, 512], f32, tag="psh")
        for q, (qoff, mq) in enumerate(qblk):
            nc.tensor.matmul(
                ps_o[:], lhsT=w3T_t[q], rhs=h2_t[q][:], start=(q == 0), stop=(q == 1)
            )
        outs = fpool.tile([48, 512], f32, tag="outs")
        nc.scalar.activation(outs[:], ps_o[:], AF.Sigmoid, bias=b3c_t)
        nc.sync.dma_start(yout_d, outs[:])

    nc.compile()
    return nc


def _shc_pack(sh):
    shc = np.zeros((128, 3), np.float32)
    shc[0:128, 0] = sh[0:128]
    shc[0:128, 1] = sh[128:256]
    shc[0:48, 2] = sh[256:304]
    return shc


def _run(maps_a, maps_b, b1, trace=False, trace_cores=None):
    from concourse.bass_utils import run_bass_kernel_spmd

    nca = _build_phase_a()
    res_a = run_bass_kernel_spmd(
        nca, maps_a, list(range(NCORES)), trace=trace, trace_cores=trace_cores
    )
    sh = np.sum([res_a.results[r]["part"][0] for r in range(NCORES)], axis=0) + b1
    shc = _shc_pack(sh)
    for mb in maps_b:
        mb["shc"] = shc
    ncb = _build_phase_b()
    res_b = run_bass_kernel_spmd(
        ncb, maps_b, list(range(NCORES)), trace=trace, trace_cores=trace_cores
    )
    full = np.empty((G * G, OUT), np.float32)
    for r in range(NCORES):
        full[512 * r : 512 * (r + 1), :] = res_b.results[r]["yout"].T
    return full.reshape(3, IMG, IMG), res_a, res_b


def kernel(**inputs):
    maps_a, maps_b, b1 = _host_inputs(**inputs)
    out, _, _ = _run(maps_a, maps_b, b1)
    return out


if __name__ == "__main__":
    import reference

    inp = {k: np.asarray(v) for k, v in reference.setup_inputs().items()}
    got = kernel(**inp)
    exp = np.asarray(reference.reference(**reference.setup_inputs()))
    err = np.abs(got - exp).max() / max(np.abs(exp).max(), 1e-9)
    print("Relative error:", err)


# revision 17
# speedup vs baseline: 1.2423x; 1.2423x over previous
"""ColorizationNet Trainium2 kernel (8 NeuronCores, SPMD, two phases).

Structure exploited: rows of the big FC input [4096, 32786] share an identical
x_conv prefix (32768 cols), so

    fc_in @ w1.T = x_conv @ w1[:, :32768].T  (one shared matvec, [304])
                 + [pos|chunks] @ w1[:, 32768:].T  ([4096,18] GEMM)

Sharding (core r of 8):
  - conv backbone row-sharded: core r produces the x_conv slice for pooled
    rows [4r, 4r+4) of every channel (halos via zero-padded input windows,
    out-of-image "phantom" rows masked to zero via activation scale).
  - shared matvec K-sharded to match (each core streams 1/8 of w1's big
    part as fp16, 2.5 MB).  Phase A outputs the 8 partials [304]; the host
    sums them (collectives work under the axon PJRT path but cost ~70 us,
    far more than the second NEFF launch they would save).
  - phase B: patch FC sharded by patch row, core r handles patches
    [512r, 512(r+1)).

Perf structure (vs the fp32 9-matmul-per-layer baseline, 82.8 us):
  - all conv/matvec/FC matmul operands fp16: moving operand streams at
    1 col/cycle at 2.4 GHz (fp32 needs 2 half-speed passes = 4x), and the
    w1 stream (phase A's DMA floor) halves to ~2.5 MB.
  - conv1's three 3x3 column taps are folded into the contraction dim
    (k=3*18=54) so each output block is ONE matmul; conv1/2/3 blocks are
    fused along the moving free dim -> 8 conv matmuls total instead of 27
    (per-matmul overhead ~250-400 ns dominates these small convs).
  - matvec k-chunks are ordered so passes 0-15 only need conv3's first
    output block (overlap with block 2), fp16 stream = 127 ns/pass.
  - DMA issue (~0.7 us of sequencer time per dma_start) spread across the
    sync/scalar/vector/gpsimd queues so descriptor generation overlaps.
"""

import sys

for _p in ("/opt/trn_rl_repo",):
    if _p not in sys.path:
        sys.path.insert(0, _p)

import numpy as np
from contextlib import ExitStack

IMG = 256
CS = 4
G = 64
H1 = 304
H2 = 176
OUT = 48
NCORES = 8

# phase-A fp16 packed stationaries: [96 partitions, 896]
#   s1f [54, 128] at cols [0, 128)   (dx-folded conv1)
#   s2  [80, 384] at cols [128, 512)
#   s3  [96, 384] at cols [512, 896)
CW16 = 896
# phase-A fp32 masks/biases: [64, 13]
#   mk1 b=0..2 @0..3, bm1 @3..6, mk2 @6..9, bm2 @9..12, bc3 @12
CW32 = 13

# phase-B fp16 packed consts: [128 partitions, 1440]
#   extrasT [18, 512] @0, w1eT [18, 304] @512, w2a/b [128, 176] @816/@992,
#   w2c [48, 176] @1168, w3a [128, 48] @1344, w3b [48, 48] @1392
CBW16 = 1440
# phase-B fp32 biases: [128, 3]: b2a [128] @0, b2b [48] @1, b3 [48] @2
CBW32 = 3


def _build_s1(c1_w):
    # [18, 3, 128]: rows i = in-row in window; cols m = s*64 + jp*8 + c
    s1 = np.zeros((18, 3, 128), np.float32)
    for dx in range(3):
        for s in range(2):
            for jp in range(8):
                j = 2 * jp + s
                for c in range(8):
                    m = s * 64 + jp * 8 + c
                    for dy in range(3):
                        s1[j + dy, dx, m] = c1_w[c, 0, dy, dx]
    return s1


def _build_s2(c2_w):
    # [80, 3, 128]: rows k = delta*8 + ci (ci in 0..8); cols m = s*64+jp*16+co
    s2 = np.zeros((80, 3, 128), np.float32)
    for dx in range(3):
        for s in range(2):
            for jp in range(4):
                j2 = 2 * jp + s
                for co in range(16):
                    m = s * 64 + jp * 16 + co
                    for ci in range(8):
                        for dy in range(3):
                            s2[(j2 + dy) * 8 + ci, dx, m] = c2_w[co, ci, dy, dx]
    return np.ascontiguousarray(s2.reshape(80, 3 * 128))


def _build_s3(c3_w):
    # [96, 3, 128]: rows k = delta*16 + ci (ci in 0..16); cols m = s*64+jpp*32+co
    s3 = np.zeros((96, 3, 128), np.float32)
    for dx in range(3):
        for s in range(2):
            for jpp in range(2):
                j3 = 2 * jpp + s
                for co in range(32):
                    m = s * 64 + jpp * 32 + co
                    for ci in range(16):
                        for dy in range(3):
                            s3[(j3 + dy) * 16 + ci, dx, m] = c3_w[co, ci, dy, dx]
    return np.ascontiguousarray(s3.reshape(96, 3 * 128))


def _host_inputs(x, c1_w, c1_b, c2_w, c2_b, c3_w, c3_b, w1, b1, w2, b2, w3, b3):
    """Returns (in_maps_a, in_maps_b_partial, b1). Each phase-A map has
    'c1mv' [54,768] f16, 'cw16' [96,896] f16, 'cw32' [64,13] f32,
    'w1ps' [128, 32*304] f16. Each phase-B map has 'cb16' [128,1440] f16 and
    'cb32' [128,3] f32; 'shc' [128,3] f32 is added after phase A."""
    f16 = np.float16
    x = np.asarray(x, np.float32).reshape(IMG, IMG)
    s1 = _build_s1(np.asarray(c1_w, np.float32))  # [18, 3, 128]
    s1f = np.ascontiguousarray(s1.transpose(1, 0, 2).reshape(54, 128))
    s2 = _build_s2(np.asarray(c2_w, np.float32))
    s3 = _build_s3(np.asarray(c3_w, np.float32))

    cw16 = np.zeros((96, CW16), f16)
    cw16[0:54, 0:128] = s1f
    cw16[0:80, 128:512] = s2
    cw16[0:96, 512:896] = s3

    # phase-B packed consts (same for every core except extrasT)
    cb16_0 = np.zeros((128, CBW16), f16)
    w1eT = np.asarray(w1, np.float32)[:, 32768:].T  # [18, 304]
    w2T = np.asarray(w2, np.float32).T  # [304, 176]
    w3T = np.asarray(w3, np.float32).T  # [176, 48]
    cb16_0[0:18, 512:816] = w1eT
    cb16_0[0:128, 816:992] = w2T[0:128]
    cb16_0[0:128, 992:1168] = w2T[128:256]
    cb16_0[0:48, 1168:1344] = w2T[256:304]
    cb16_0[0:128, 1344:1392] = w3T[0:128]
    cb16_0[0:48, 1392:1440] = w3T[128:176]
    cb32 = np.zeros((128, CBW32), np.float32)
    cb32[0:128, 0] = np.asarray(b2, np.float32)[0:128]
    cb32[0:48, 1] = np.asarray(b2, np.float32)[128:176]
    cb32[0:48, 2] = np.asarray(b3, np.float32)

    w1bigT = np.ascontiguousarray(np.asarray(w1, np.float32)[:, :32768].T)  # [32768, 304]
    chunks = x.reshape(G, CS, G, CS).transpose(0, 2, 1, 3).reshape(G * G, CS * CS)
    pi = (np.arange(G * G) // G).astype(np.float32) * CS
    pj = (np.arange(G * G) % G).astype(np.float32) * CS

    c1b = np.asarray(c1_b, np.float32)
    c2b = np.asarray(c2_b, np.float32)
    bc3v = np.tile(np.asarray(c3_b, np.float32), 2)  # [64]

    # matvec k-chunk layout: pass b = 16*g + b' uses conv3 output block g;
    # partition p = 64*hi + jpp*32 + co -> x_conv (co, 4r + 2g + jpp, 16*hi + b')
    P = np.arange(128)[:, None]
    B = np.arange(32)[None, :]
    co = P % 32
    jpp = (P % 64) // 32
    hi = P // 64
    g = B // 16
    bp = B % 16

    maps_a, maps_b = [], []
    for r in range(NCORES):
        # padded input strip: x rows [32r-7, 32r+43), cols padded by 1
        xpad = np.zeros((50, 258), np.float32)
        lo = 32 * r - 7
        hi_row = 32 * r + 43
        slo, shi = max(lo, 0), min(hi_row, IMG)
        xpad[slo - lo : shi - lo, 1:257] = x[slo:shi, :]

        # conv1 moving, dx folded into partitions, 3 row-blocks along free
        c1mv = np.zeros((54, 768), f16)
        for dx in range(3):
            for b in range(3):
                c1mv[dx * 18 : dx * 18 + 18, 256 * b : 256 * b + 256] = xpad[
                    16 * b : 16 * b + 18, dx : dx + 256
                ]

        cw32 = np.zeros((64, CW32), np.float32)
        for b in range(3):
            for jp in range(8):
                valid = 0 <= (16 * r - 3 + 8 * b + jp) < 128
                cw32[jp * 8 : jp * 8 + 8, 0 + b] = 1.0 if valid else 0.0
                cw32[jp * 8 : jp * 8 + 8, 3 + b] = c1b if valid else 0.0
            for jp in range(4):
                valid = 0 <= (8 * r - 1 + 4 * b + jp) < 64
                cw32[jp * 16 : jp * 16 + 16, 6 + b] = 1.0 if valid else 0.0
                cw32[jp * 16 : jp * 16 + 16, 9 + b] = c2b if valid else 0.0
        cw32[0:64, 12] = bc3v

        row = 4 * r + 2 * g + jpp
        kg = co * 1024 + row * 32 + 16 * hi + bp  # [128, 32]
        w1ps = w1bigT[kg.ravel()].reshape(128, 32 * 304).astype(f16)
        maps_a.append({"c1mv": c1mv, "cw16": cw16, "cw32": cw32, "w1ps": w1ps})

        cb16 = cb16_0.copy()
        sl = slice(512 * r, 512 * (r + 1))
        cb16[0, 0:512] = pi[sl]
        cb16[1, 0:512] = pj[sl]
        cb16[2:18, 0:512] = chunks[sl].T
        maps_b.append({"cb16": cb16, "cb32": cb32})
    return maps_a, maps_b, np.asarray(b1, np.float32)


def _mk_nc():
    import concourse.bacc as bacc

    # Bacc (not raw Bass): its compile() runs move_matmul_waits_to_ldweights /
    # generate_event_semaphores, required for the 1-wait-per-instruction
    # hardware constraint.
    return bacc.Bacc("TRN2", target_bir_lowering=False, debug=False, num_devices=NCORES)


def _build_phase_a():
    """Convs + sharded shared-matvec partial. Output: part [1, 304]."""
    import concourse.tile as tile
    from concourse import mybir

    f32 = mybir.dt.float32
    f16 = mybir.dt.float16
    AF = mybir.ActivationFunctionType
    nc = _mk_nc()

    c1mv_d = nc.dram_tensor("c1mv", [54, 768], f16, kind="ExternalInput").ap()
    cw16_d = nc.dram_tensor("cw16", [96, CW16], f16, kind="ExternalInput").ap()
    cw32_d = nc.dram_tensor("cw32", [64, CW32], f32, kind="ExternalInput").ap()
    w1ps_d = nc.dram_tensor("w1ps", [128, 32 * 304], f16, kind="ExternalInput").ap()
    part_d = nc.dram_tensor("part", [1, 304], f32, kind="ExternalOutput").ap()

    with tile.TileContext(nc) as tc, ExitStack() as ctx:
        cpool = ctx.enter_context(tc.tile_pool(name="consts", bufs=1))
        spool = ctx.enter_context(tc.tile_pool(name="work", bufs=1))
        pconv = ctx.enter_context(tc.tile_pool(name="pconv", bufs=1, space="PSUM"))
        pmv = ctx.enter_context(tc.tile_pool(name="pmv", bufs=1, space="PSUM"))

        # warm the ScalarE activation-function table early (overlaps DMAs)
        scr = cpool.tile([1, 1], f32, tag="scr")
        nc.vector.memset(scr[:], 0.0)
        scr2 = cpool.tile([1, 1], f32, tag="scr2")
        nc.scalar.activation(scr2[:], scr[:], AF.Relu)

        # All DMAs on the sync HWDGE queue, in criticality order — descriptor
        # FIFO order in the shared DMA rings follows issue order, so the small
        # conv inputs must be fully issued BEFORE the 2.5 MB w1 stream or they
        # queue behind it (+7 us to conv1's first matmul). ScalarE's queue
        # stays DMA-free: its descriptor generation stalls the conv RELUs.
        cw16_t = cpool.tile([96, CW16], f16, tag="cw16")
        nc.sync.dma_start(cw16_t[:], cw16_d)
        c1mv_t = cpool.tile([54, 768], f16, tag="c1mv")
        nc.sync.dma_start(c1mv_t[:], c1mv_d)
        cw32_t = cpool.tile([64, CW32], f32, tag="cw32")
        nc.scalar.dma_start(cw32_t[:], cw32_d)
        wst = cpool.tile([128, 32 * 304], f16, tag="w1s")
        half = 16 * 304
        nc.sync.dma_start(wst[:, 0:half], w1ps_d[:, 0:half])
        nc.sync.dma_start(wst[:, half : 2 * half], w1ps_d[:, half : 2 * half])

        s1f = cw16_t[0:54, 0:128]
        s2ap = lambda dx: cw16_t[0:80, 128 + 128 * dx : 256 + 128 * dx]
        s3ap = lambda dx: cw16_t[0:96, 512 + 128 * dx : 640 + 128 * dx]
        mk1 = lambda b, n=64: cw32_t[0:n, 0 + b : 1 + b]
        bm1 = lambda b, n=64: cw32_t[0:n, 3 + b : 4 + b]
        mk2 = lambda b, n=64: cw32_t[0:n, 6 + b : 7 + b]
        bm2 = lambda b, n=64: cw32_t[0:n, 9 + b : 10 + b]
        bc3 = cw32_t[0:64, 12:13]

        # next-layer moving-window tiles (built in place by ScalarE writes)
        m2all = cpool.tile([80, 390], f16, tag="m2all")  # 3 x (1+128+1)
        m3all = cpool.tile([96, 132], f16, tag="m3all")  # 2 x (1+64+1)
        xc_t = cpool.tile([128, 32], f16, tag="xc")
        nc.vector.memset(m2all[:], 0.0)
        nc.vector.memset(m3all[:], 0.0)

        def vpool(ps, width, tag):
            """psum [128, width] (partition = (s, *)) -> [64, width] s-max."""
            vtop = spool.tile([64, width], f32, tag=f"vt{tag}")
            nc.scalar.copy(vtop[:], ps[0:64, :])
            v = spool.tile([64, width], f32, tag=f"v{tag}")
            nc.vector.tensor_max(v[:], ps[64:128, :], vtop[:])
            return v

        def hpool(v, width, tag):
            """[64, width] -> [64, width//2] adjacent-pair max."""
            vv = v[:].rearrange("p (x t) -> p x t", t=2)
            ph = spool.tile([64, width // 2], f32, tag=f"ph{tag}")
            nc.vector.tensor_max(ph[:], vv[:, :, 0], vv[:, :, 1])
            return ph

        # ---- conv1: dx folded in k; blocks 0,1 fused (n=512) + block 2
        ps1a = pconv.tile([128, 512], f32, tag="ps1a")
        nc.tensor.matmul(ps1a[:], lhsT=s1f, rhs=c1mv_t[:, 0:512], start=True, stop=True)
        ps1b = pconv.tile([128, 256], f32, tag="ps1b")
        nc.tensor.matmul(ps1b[:], lhsT=s1f, rhs=c1mv_t[:, 512:768], start=True, stop=True)
        ph1a = hpool(vpool(ps1a, 512, "1a"), 512, "1a")  # [64, 256] = (b 2, x 128)
        ph1b = hpool(vpool(ps1b, 256, "1b"), 256, "1b")  # [64, 128]
        A = nc.scalar.activation
        A(m2all[0:64, 1:129], ph1a[:, 0:128], AF.Relu, bias=bm1(0), scale=mk1(0))
        A(m2all[0:64, 131:259], ph1a[:, 128:256], AF.Relu, bias=bm1(1), scale=mk1(1))
        A(m2all[0:64, 261:389], ph1b[:], AF.Relu, bias=bm1(2), scale=mk1(2))
        A(m2all[64:80, 1:129], ph1a[0:16, 128:256], AF.Relu, bias=bm1(1, 16), scale=mk1(1, 16))
        A(m2all[64:80, 131:259], ph1b[0:16, :], AF.Relu, bias=bm1(2, 16), scale=mk1(2, 16))

        # ---- conv2: 3 dx-matmuls, blocks fused (n=384)
        ps2 = pconv.tile([128, 384], f32, tag="ps2")
        m2v = m2all[:].rearrange("p (b c) -> p b c", b=3)
        for dx in range(3):
            nc.tensor.matmul(
                ps2[:], lhsT=s2ap(dx), rhs=m2v[:, :, dx : dx + 128],
                start=(dx == 0), stop=(dx == 2),
            )
        ph2 = hpool(vpool(ps2, 384, "2"), 384, "2")  # [64, 192] = (b 3, x 64)
        A(m3all[0:64, 1:65], ph2[:, 0:64], AF.Relu, bias=bm2(0), scale=mk2(0))
        A(m3all[0:64, 67:131], ph2[:, 64:128], AF.Relu, bias=bm2(1), scale=mk2(1))
        A(m3all[64:96, 1:65], ph2[0:32, 64:128], AF.Relu, bias=bm2(1, 32), scale=mk2(1, 32))
        A(m3all[64:96, 67:131], ph2[0:32, 128:192], AF.Relu, bias=bm2(2, 32), scale=mk2(2, 32))

        # ---- conv3: 3 dx-matmuls, g-blocks fused (n=128)
        ps3 = pconv.tile([128, 128], f32, tag="ps3")
        m3v = m3all[:].rearrange("p (g c) -> p g c", g=2)
        for dx in range(3):
            nc.tensor.matmul(
                ps3[:], lhsT=s3ap(dx), rhs=m3v[:, :, dx : dx + 64],
                start=(dx == 0), stop=(dx == 2),
            )
        ph3 = hpool(vpool(ps3, 128, "3"), 128, "3")  # [64, 64] = (g 2, x 32)
        # xc[64*hi:, 16g:16g+16] = relu(ph3[:, 32g+16hi : +16] + bc3)
        A(xc_t[0:64, 0:16], ph3[:, 0:16], AF.Relu, bias=bc3)
        A(xc_t[64:128, 0:16], ph3[:, 16:32], AF.Relu, bias=bc3)
        A(xc_t[0:64, 16:32], ph3[:, 32:48], AF.Relu, bias=bc3)
        A(xc_t[64:128, 16:32], ph3[:, 48:64], AF.Relu, bias=bc3)

        # ---- shared matvec partial [1, 304]; passes 0-15 need only g=0
        ps_mv = pmv.tile([1, 304], f32, tag="mv")
        for b in range(32):
            nc.tensor.matmul(
                ps_mv[:],
                lhsT=xc_t[:, b : b + 1],
                rhs=wst[:, 304 * b : 304 * (b + 1)],
                start=(b == 0),
                stop=(b == 31),
            )
        part_s = spool.tile([1, 304], f32, tag="part")
        nc.scalar.copy(part_s[:], ps_mv[:])
        nc.sync.dma_start(part_d, part_s[:])

    nc.compile()
    return nc


def _build_phase_b():
    """Patch FC for this core's 512 patches, given summed shared vector."""
    import concourse.tile as tile
    from concourse import mybir

    f32 = mybir.dt.float32
    f16 = mybir.dt.float16
    AF = mybir.ActivationFunctionType
    ALU = mybir.AluOpType
    nc = _mk_nc()

    cb16_d = nc.dram_tensor("cb16", [128, CBW16], f16, kind="ExternalInput").ap()
    cb32_d = nc.dram_tensor("cb32", [128, CBW32], f32, kind="ExternalInput").ap()
    shc_d = nc.dram_tensor("shc", [128, 3], f32, kind="ExternalInput").ap()
    yout_d = nc.dram_tensor("yout", [48, 512], f32, kind="ExternalOutput").ap()

    mblk = [(0, 128), (128, 128), (256, 48)]
    qblk = [(0, 128), (128, 48)]

    with tile.TileContext(nc) as tc, ExitStack() as ctx:
        cpool = ctx.enter_context(tc.tile_pool(name="consts", bufs=1))
        fpool = ctx.enter_context(tc.tile_pool(name="fc", bufs=1))
        pfc = ctx.enter_context(tc.tile_pool(name="pfc", bufs=1, space="PSUM"))
        phh = ctx.enter_context(tc.tile_pool(name="phh", bufs=3, space="PSUM"))

        # warm the ScalarE Sigmoid table early (overlaps DMAs); ReLUs are on DVE
        scr = cpool.tile([1, 1], f32, tag="scr")
        nc.vector.memset(scr[:], 0.0)
        scr2 = cpool.tile([1, 1], f32, tag="scr2")
        nc.scalar.activation(scr2[:], scr[:], AF.Sigmoid)

        cb = cpool.tile([128, CBW16], f16, tag="cb16")
        nc.sync.dma_start(cb[:, 0:816], cb16_d[:, 0:816])
        shc = cpool.tile([128, 3], f32, tag="shc")
        nc.scalar.dma_start(shc[:], shc_d)
        cbb = cpool.tile([128, CBW32], f32, tag="cb32")
        nc.scalar.dma_start(cbb[:], cb32_d)
        nc.sync.dma_start(cb[:, 816:CBW16], cb16_d[:, 816:CBW16])

        extrasT = cb[0:18, 0:512]
        w1eT = cb[0:18, 512:816]
        w2T_t = [cb[0:128, 816:992], cb[0:128, 992:1168], cb[0:48, 1168:1344]]
        w3T_t = [cb[0:128, 1344:1392], cb[0:48, 1392:1440]]
        b2c_t = [cbb[0:128, 0:1], cbb[0:48, 1:2]]
        b3c_t = cbb[0:48, 2:3]
        sh_t = [shc[0:128, 0:1], shc[0:128, 1:2], shc[0:48, 2:3]]

        h1_t = []
        for i, (off, mb) in enumerate(mblk):
            ps_e = pfc.tile([mb, 512], f32, tag=f"pse{i}")
            nc.tensor.matmul(
                ps_e[:], lhsT=w1eT[:, off : off + mb], rhs=extrasT, start=True, stop=True
            )
            h1 = fpool.tile([mb, 512], f16, tag=f"h1{i}")
            nc.vector.tensor_scalar(h1[:], ps_e[:], sh_t[i], 0.0, ALU.add, ALU.max)
            h1_t.append(h1)

        h2_t = []
        for q, (qoff, mq) in enumerate(qblk):
            ps_h = phh.tile([mq, 512], f32, tag="psh")
            for i, (off, mb) in enumerate(mblk):
                nc.tensor.matmul(
                    ps_h[:],
                    lhsT=w2T_t[i][:, qoff : qoff + mq],
                    rhs=h1_t[i][:],
                    start=(i == 0),
                    stop=(i == 2),
                )
            h2 = fpool.tile([mq, 512], f16, tag=f"h2{q}")
            nc.vector.tensor_scalar(h2[:], ps_h[:], b2c_t[q], 0.0, ALU.add, ALU.max)
            h2_t.append(h2)

        ps_o = phh.tile([48, 512], f32, tag="psh")
        for q, (qoff, mq) in enumerate(qblk):
            nc.tensor.matmul(
                ps_o[:], lhsT=w3T_t[q], rhs=h2_t[q][:], start=(q == 0), stop=(q == 1)
            )
        outs = fpool.tile([48, 512], f32, tag="outs")
        nc.scalar.activation(outs[:], ps_o[:], AF.Sigmoid, bias=b3c_t)
        nc.sync.dma_start(yout_d, outs[:])

    nc.compile()
    return nc


def _shc_pack(sh):
    shc = np.zeros((128, 3), np.float32)
    shc[0:128, 0] = sh[0:128]
    shc[0:128, 1] = sh[128:256]
    shc[0:48, 2] = sh[256:304]
    return shc


def _run(maps_a, maps_b, b1, trace=False, trace_cores=None):
    from concourse.bass_utils import run_bass_kernel_spmd

    nca = _build_phase_a()
    res_a = run_bass_kernel_spmd(
        nca, maps_a, list(range(NCORES)), trace=trace, trace_cores=trace_cores
    )
    sh = np.sum([res_a.results[r]["part"][0] for r in range(NCORES)], axis=0) + b1
    shc = _shc_pack(sh)
    for mb in maps_b:
        mb["shc"] = shc
    ncb = _build_phase_b()
    res_b = run_bass_kernel_spmd(
        ncb, maps_b, list(range(NCORES)), trace=trace, trace_cores=trace_cores
    )
    full = np.empty((G * G, OUT), np.float32)
    for r in range(NCORES):
        full[512 * r : 512 * (r + 1), :] = res_b.results[r]["yout"].T
    return full.reshape(3, IMG, IMG), res_a, res_b


def kernel(**inputs):
    maps_a, maps_b, b1 = _host_inputs(**inputs)
    out, _, _ = _run(maps_a, maps_b, b1)
    return out


if __name__ == "__main__":
    import reference

    inp = {k: np.asarray(v) for k, v in reference.setup_inputs().items()}
    got = kernel(**inp)
    exp = np.asarray(reference.reference(**reference.setup_inputs()))
    err = np.abs(got - exp).max() / max(np.abs(exp).max(), 1e-9)
    print("Relative error:", err)
